# revision 44
# baseline (speedup 1.0000x reference)
"""CrossAgentAttention Trainium2 kernel.

Problem: B=1024 samples, N=32 agents, D=512 features, H=4 heads (HD=128).
  qkv = x @ Win^T + bin ; per-head attention over the N=32 agents with the
  diagonal (self) and padded agents masked out of the keys; out = ctx @ Wout^T + bout.

Strategy (data-parallel over B across 8 cores, weights replicated):
  - Host pre-transposes the per-core activations to X^T [D, T] (T = B/8*32 tokens)
    and the weights to Win^T / Wout^T so every GEMM contraction dim lands on
    SBUF partitions.  Q columns of Win^T are pre-scaled by 1/sqrt(HD).
  - Stage 1: Q^T,K^T [1024, T] in transposed (feature-major) layout and
    V [T, 512] token-major, via f32r matmuls with N=512 moving operands.
  - Stage 2: attention per (sample-group of 4, head).  128 tokens = 4 samples
    x 32 agents: S = Q^T.T @ K^T gives all 16 cross-sample blocks; an additive
    mask kills cross-sample blocks, the self-diagonal, and padded keys.
    Softmax without max-subtraction (logits are O(1) by construction),
    exp+rowsum fused on ACT, P normalized on DVE, P^T via PE transpose,
    ctx^T = (V slice).T @ P^T.
  - Stage 3: OUT^T = Wout^T.T @ ctx^T, DMA out; host transposes back.
"""

import math

import ml_dtypes
import numpy as np

import concourse.bass as bass
import concourse.mybir as mybir
import concourse.tile as tile
from concourse import bacc
from concourse.bass_utils import run_bass_kernel_spmd

N_CORES = 8
B, N, D, H = 1024, 32, 512, 4
HD = D // H  # 128
NEG = -60000.0  # additive mask value; exp() underflows to exactly 0.0
F32 = mybir.dt.float32
F32R = mybir.dt.float32r
BF16 = mybir.dt.bfloat16

_MM_DT = {"f32r": F32R, "f32": F32, "bf16": BF16}


def build_program(b_core, mm_dtype="bf16", reps=1, with_pad=False,
                  with_bias=False, skel=False):
    """Trace + compile the per-core program. Returns (nc, meta)."""
    T = b_core * N  # tokens per core
    GT = 512 if T >= 512 else T  # tokens per group
    G = T // GT  # groups
    TT = GT // 128  # 128-token tiles (sample groups of 4) per group
    assert T % 512 == 0 or G == 1

    nc = bacc.Bacc("TRN2", target_bir_lowering=False, debug=False, num_devices=N_CORES)

    MD = _MM_DT[mm_dtype]  # matmul-operand dtype
    OD = BF16 if mm_dtype == "bf16" else F32  # DRAM output dtype
    # [g, p, k, c] = X^T[k*128+p, g*GT+c]; one contiguous DMA per group
    # (4*GT*elem per partition line) instead of 4 strided 128-line loads.
    xt = nc.dram_tensor("xt", [G, 128, 4, GT], MD, kind="ExternalInput").ap()
    wint = nc.dram_tensor("wint", [D, 3 * D], MD, kind="ExternalInput").ap()
    woutt = nc.dram_tensor("woutt", [D, D], MD, kind="ExternalInput").ap()
    ident = nc.dram_tensor("ident", [128, 128], MD, kind="ExternalInput").ap()
    if with_pad:
        mask = nc.dram_tensor("mask", [T // 128, 128, 4 * 128], F32,
                              kind="ExternalInput").ap()
    else:
        mask = nc.dram_tensor("mask", [128, TT, 4 * 128], F32,
                              kind="ExternalInput").ap()
    if with_bias:
        bqk = nc.dram_tensor("bqk", [128, 8], F32, kind="ExternalInput").ap()
        bv = nc.dram_tensor("bv", [1, D], MD, kind="ExternalInput").ap()
        bo = nc.dram_tensor("bo", [128, 4], F32, kind="ExternalInput").ap()
    # [g, p, fo, c] = OUT^T[fo*128+p, g*GT+c]; one contiguous DMA per group
    outt = nc.dram_tensor("outt", [G, 128, 4, GT], OD, kind="ExternalOutput").ap()

    assert TT % 2 == 0
    NP = TT // 2  # attention tile-pairs per group

    with tile.TileContext(nc) as tc:
        with (
            tc.tile_pool(name="wpool", bufs=1) as wpool,
            tc.tile_pool(name="xtp", bufs=2, space="SBUF") as xtp,
            tc.tile_pool(name="qktp", bufs=2 * 8) as qktp,
            tc.tile_pool(name="vp", bufs=2 * TT) as vp,
            tc.tile_pool(name="smp", bufs=2) as smp,
            tc.tile_pool(name="ctxp", bufs=2) as ctxp,
            tc.tile_pool(name="otp", bufs=2) as otp,
            tc.tile_pool(name="mmps", bufs=2, space="PSUM") as mmps,
            tc.tile_pool(name="spsp", bufs=1, space="PSUM") as spsp,
            tc.tile_pool(name="ptpsp", bufs=1, space="PSUM") as ptpsp,
        ):
            # ---- resident weights / constants ----
            w = []
            for k in range(4):
                wt = wpool.tile([128, 3 * D], MD, tag=f"wint{k}")
                w.append(wt)
            # chunked so Q columns (chunk 0) land first; K then V follow
            for c in range(3):
                for k in range(4):
                    nc.sync.dma_start(
                        w[k][:, bass.ts(c, D)],
                        wint[k * 128:(k + 1) * 128, bass.ts(c, D)])
            idt = wpool.tile([128, 128], MD, tag="ident")
            nc.sync.dma_start(idt[:], ident[:])
            mk_const = None
            if not with_pad:
                mk_const = wpool.tile([128, TT, 4 * 128], F32, tag="mask")
                nc.sync.dma_start(mk_const[:], mask[:])
            wo = []
            for k in range(4):
                wt = wpool.tile([128, D], MD, tag=f"woutt{k}")
                nc.sync.dma_start(wt[:], woutt[k * 128:(k + 1) * 128, :])
                wo.append(wt)
            if with_bias:
                bqk_sb = wpool.tile([128, 8], F32, tag="bqk")
                nc.sync.dma_start(bqk_sb[:], bqk[:])
                bv_sb = wpool.tile([1, D], MD, tag="bv")
                nc.sync.dma_start(bv_sb[:], bv[:])
                bo_sb = wpool.tile([128, 4], F32, tag="bo")
                nc.sync.dma_start(bo_sb[:], bo[:])
                ones_sb = wpool.tile([1, 128], MD, tag="ones")
                nc.vector.memset(ones_sb[:], 1.0)

            def body(_iv=None):
                def emit_load(g):
                    xgt = xtp.tile([128, 4, GT], MD, tag="xt", name="xgt")
                    nc.scalar.dma_start(xgt[:], xt[g])
                    return xgt

                def emit_1a(xgt):
                    # Q^T, K^T (feature-major)
                    qkt = []
                    for fo in range(8):
                        ps = mmps.tile([128, GT], F32, tag="mm", name="ps")
                        for k in range(4):
                            nc.tensor.matmul(
                                ps[:],
                                w[k][:, bass.ts(fo, 128)],
                                xgt[:, k, :],
                                start=(k == 0), stop=(k == 3),
                            )
                        qt = qktp.tile([128, GT], MD, tag="qkt", name="qt")
                        if with_bias:
                            nc.scalar.activation(
                                qt[:], ps[:],
                                mybir.ActivationFunctionType.Identity,
                                bias=bqk_sb[:, fo:fo + 1])
                        else:
                            nc.vector.tensor_copy(qt[:], ps[:])
                        qkt.append(qt)
                    return qkt

                def emit_1b(xgt):
                    # V (token-major)
                    vg = []
                    for tt in range(TT):
                        ps = mmps.tile([128, D], F32, tag="mm", name="ps")
                        for k in range(4):
                            nc.tensor.matmul(
                                ps[:],
                                xgt[:, k, bass.ts(tt, 128)],
                                w[k][:, 2 * D:3 * D],
                                start=(k == 0),
                                stop=(k == 3 and not with_bias),
                            )
                        if with_bias:
                            nc.tensor.matmul(
                                ps[:], ones_sb[:], bv_sb[:],
                                start=False, stop=True,
                            )
                        vt = vp.tile([128, D], MD, tag="v", name="vt")
                        nc.scalar.copy(vt[:], ps[:])
                        vg.append(vt)
                    return vg

                ot_holder = [None]

                def emit_outproj(ctxt_prev, g_prev, half=None):
                    halves = (0, 1) if half is None else (half,)
                    for hf in halves:
                        if hf == 0:
                            ot_holder[0] = otp.tile([128, 4, GT], OD,
                                                    tag="ot", name="ot")
                        ot = ot_holder[0]
                        for fo in (2 * hf, 2 * hf + 1):
                            ps = mmps.tile([128, GT], F32, tag="mm", name="ps")
                            for k in range(4):
                                nc.tensor.matmul(
                                    ps[:],
                                    wo[k][:, bass.ts(fo, 128)],
                                    ctxt_prev[:, k, :],
                                    start=(k == 0), stop=(k == 3),
                                )
                            if with_bias:
                                nc.scalar.activation(
                                    ot[:, fo, :], ps[:],
                                    mybir.ActivationFunctionType.Identity,
                                    bias=bo_sb[:, fo:fo + 1])
                            else:
                                nc.scalar.copy(ot[:, fo, :], ps[:])
                        if hf == 1:
                            nc.sync.dma_start(outt[g_prev], ot[:])

                if skel:
                    pending = None
                    for g in range(G):
                        xgt = emit_load(g)
                        qkt = emit_1a(xgt)
                        emit_1b(xgt)
                        if pending is not None:
                            emit_outproj(*pending)
                        ctxt = ctxp.tile([128, 4, GT], MD, tag="ctxt",
                                         name="ctxt")
                        for k in range(4):
                            nc.vector.tensor_copy(ctxt[:, k, :], qkt[k][:])
                        pending = (ctxt, g)
                    emit_outproj(*pending)
                    return

                # ---- group-level software pipeline: group g's attention
                # overlaps group g+1's QKV projections (PE fill while the
                # softmax chain drains on ACT/DVE). exp(S+M) == exp(S)*Z
                # with Z in {0,1}: exp reads raw S straight from PSUM, mask
                # folded into the DVE pipeline as a multiply.
                def emit_S(g, qkt):
                    sps = spsp.tile([128, TT, 4, 128], F32, tag="sps",
                                    name="sps")
                    if with_pad:
                        mk = smp.tile([128, TT, 4 * 128], F32, tag="mask",
                                      name="mk")
                        for t in range(TT):
                            nc.sync.dma_start(mk[:, t, :], mask[g * TT + t])
                        zm = mk
                    else:
                        zm = mk_const
                    for t in range(TT):
                        tsl = bass.ts(t, 128)
                        for h in range(4):
                            nc.tensor.matmul(
                                sps[:, t, h, :],
                                qkt[h][:, tsl],
                                qkt[4 + h][:, tsl],
                                start=True, stop=True,
                            )
                    return sps, zm

                def emit_softmax(sps, zm):
                    psb = smp.tile([128, TT, 4, 128], F32, tag="psb",
                                   name="psb")
                    nc.scalar.activation(
                        psb[:], sps[:], mybir.ActivationFunctionType.Exp)
                    pz = smp.tile([128, TT, 4, 128], F32, tag="pz", name="pz")
                    nc.vector.tensor_mul(
                        pz[:].rearrange("p t h j -> p t (h j)"),
                        psb[:].rearrange("p t h j -> p t (h j)"),
                        zm[:])
                    rsum = smp.tile([128, 2 * 4 * TT], F32, tag="rsum",
                                    name="rsum")
                    nt = 4 * TT
                    nc.vector.reduce_sum(
                        rsum[:, 0:nt],
                        pz[:].rearrange("p t h j -> p (t h) j"),
                        axis=mybir.AxisListType.X)
                    nc.vector.reciprocal(rsum[:, nt:2 * nt], rsum[:, 0:nt])
                    pnb = smp.tile([128, TT, 4, 128], MD, tag="pnb",
                                   name="pnb")
                    rb = rsum[:, nt:2 * nt]
                    rinv_b = bass.AP(tensor=rb.tensor, offset=rb.offset,
                                     ap=list(rb.ap) + [[0, 128]])
                    nc.vector.tensor_mul(
                        pnb[:].rearrange("p t h j -> p (t h) j"),
                        pz[:].rearrange("p t h j -> p (t h) j"),
                        rinv_b)
                    return pnb

                def emit_T(pnb):
                    ptps = ptpsp.tile([128, TT, 4, 128], MD, tag="ptps",
                                      name="ptps")
                    for t in range(TT):
                        for h in range(4):
                            nc.tensor.transpose(
                                ptps[:, t, h, :], pnb[:, t, h, :], idt[:])
                    return ptps

                def emit_ptsb(ptps, u):
                    # copy one pair-half of P^T out of PSUM
                    ptsb = smp.tile([128, 2, 4, 128], MD, tag=f"ptsb{u}",
                                    name="ptsb")
                    nc.scalar.copy(ptsb[:], ptps[:, 2 * u:2 * u + 2, :, :])
                    return ptsb

                def emit_ctx(ctxt, vg, ptsb, tt):
                    cps = mmps.tile([128, 4, 128], F32, tag="mm", name="cps")
                    for h in range(4):
                        nc.tensor.matmul(
                            cps[:, h, :],
                            vg[tt][:, bass.ts(h, 128)],
                            ptsb[:, tt % 2, h, :],
                            start=True, stop=True,
                        )
                    nc.scalar.copy(
                        ctxt[:, :, bass.ts(tt, 128)],
                        cps[:])

                # prologue: first group's projections
                xgt = emit_load(0)
                qkts = {0: emit_1a(xgt)}
                vgs = {0: emit_1b(xgt)}
                pending = None
                for g in range(G):
                    if g + 1 < G:
                        xgt = emit_load(g + 1)
                    qkt = qkts.pop(g)
                    vg = vgs.pop(g)
                    sps, zm = emit_S(g, qkt)
                    pnb = emit_softmax(sps, zm)
                    fill_done = False
                    if g + 1 < G:
                        qkts[g + 1] = emit_1a(xgt)   # PE fill: softmax drain
                        vgs[g + 1] = emit_1b(xgt)
                    elif pending is not None:
                        emit_outproj(*pending)       # last group: prev outproj
                        fill_done = True
                    ptps = emit_T(pnb)
                    ptsb0 = emit_ptsb(ptps, 0)
                    ptsb1 = emit_ptsb(ptps, 1)
                    ctxt = ctxp.tile([128, 4, GT], MD, tag="ctxt", name="ctxt")
                    if pending is not None and not fill_done:
                        emit_outproj(*pending, half=0)   # PE fill: ptsb copy
                    emit_ctx(ctxt, vg, ptsb0, 0)
                    emit_ctx(ctxt, vg, ptsb0, 1)
                    if pending is not None and not fill_done:
                        emit_outproj(*pending, half=1)
                    emit_ctx(ctxt, vg, ptsb1, 2)
                    emit_ctx(ctxt, vg, ptsb1, 3)
                    pending = (ctxt, g)
                emit_outproj(*pending)

            if reps == 1:
                body()
            else:
                with tc.For_i(0, reps, 1, hint_engines=(
                        mybir.EngineType.PE, mybir.EngineType.DVE,
                        mybir.EngineType.Activation, mybir.EngineType.SP)) as iv:
                    body(iv)

    nc.compile()
    return nc


def make_host_inputs(agent_hiddens, padding_mask, in_proj_weight, in_proj_bias,
                     out_proj_weight, out_proj_bias, mm_dtype="bf16"):
    """Shard + lay out host-side numpy arrays. Returns (in_maps, flags)."""
    hd = ml_dtypes.bfloat16 if mm_dtype == "bf16" else np.float32
    x = np.asarray(agent_hiddens, dtype=np.float32)
    pad = np.asarray(padding_mask)
    win = np.asarray(in_proj_weight, dtype=np.float32)
    bin_ = np.asarray(in_proj_bias, dtype=np.float32)
    wout = np.asarray(out_proj_weight, dtype=np.float32)
    bout = np.asarray(out_proj_bias, dtype=np.float32)

    b = x.shape[0]
    b_core = b // N_CORES
    T = b_core * N
    scale = 1.0 / math.sqrt(HD)

    with_pad = bool(pad.any())
    with_bias = bool(bin_.any() or bout.any())

    wint = np.ascontiguousarray(win.T)
    wint[:, :D] *= scale
    wint = wint.astype(hd)
    woutt = np.ascontiguousarray(wout.T).astype(hd)
    identity = np.eye(128, dtype=hd)

    # 128-token block mask: tokens (s, i) x (s', j); mask cross-sample blocks
    # and the global diagonal (self-attention). The kernel consumes 0/1
    # multiplicative masks (applied after exp).
    p = np.arange(128)
    blocked = ((p[:, None] // 32 != p[None, :] // 32)
               | (p[:, None] == p[None, :]))
    mask_tile = np.tile(np.where(blocked, 0.0, 1.0).astype(np.float32),
                        (1, 4))  # [128, 512], head-replicated

    G = T // 512 if T >= 512 else 1
    GT = T // G
    in_maps = []
    for c in range(N_CORES):
        xc = x[c * b_core:(c + 1) * b_core].reshape(T, D)
        # [G, 128, 4, GT]: [g, p, k, c] = X^T[k*128+p, g*GT+c]
        xct = xc.T.reshape(4, 128, G, GT).transpose(2, 1, 0, 3)
        m = {
            "xt": np.ascontiguousarray(xct).astype(hd),
            "wint": wint,
            "woutt": woutt,
            "ident": identity,
        }
        if with_pad:
            padc = pad[c * b_core:(c + 1) * b_core]  # [b_core, N]
            n_tt = T // 128
            mt = np.empty((n_tt, 128, 512), dtype=np.float32)
            for t in range(n_tt):
                # 4 samples in this tile; key-padding kills columns
                pr = padc[t * 4:(t + 1) * 4].reshape(128)  # [(s', j)] order
                keep = (~blocked) & (~pr[None, :])
                mt[t] = np.tile(keep.astype(np.float32), (1, 4))
            m["mask"] = mt
        else:
            # [128, TT, 512]: same per-tile mask for every tile of a group
            TT = GT // 128
            m["mask"] = np.ascontiguousarray(
                np.broadcast_to(mask_tile[:, None, :], (128, TT, 512)))
        if with_bias:
            bq = bin_[:D] * scale
            bk = bin_[D:2 * D]
            m["bqk"] = np.ascontiguousarray(
                np.concatenate([bq, bk]).reshape(8, 128).T)
            m["bv"] = bin_[2 * D:3 * D].reshape(1, D).astype(hd)
            m["bo"] = np.ascontiguousarray(bout.reshape(4, 128).T)
        in_maps.append(m)
    return in_maps, dict(b_core=b_core, with_pad=with_pad, with_bias=with_bias)


def assemble_output(results, b_core):
    outs = []
    for c in range(N_CORES):
        ot = np.asarray(results[c]["outt"], dtype=np.float32)  # [G,128,4,GT]
        # [g, p, fo, c] = OUT^T[fo*128+p, g*GT+c] -> [token, feat]
        G = ot.shape[0]
        out = ot.transpose(0, 3, 2, 1).reshape(b_core * N, D)
        outs.append(out.reshape(b_core, N, D))
    return np.ascontiguousarray(np.concatenate(outs, axis=0))


_NC_CACHE = {}


def _get_nc(key_args):
    key = tuple(sorted(key_args.items()))
    if key not in _NC_CACHE:
        _NC_CACHE[key] = build_program(**key_args)
    return _NC_CACHE[key]


MM_DTYPE = "bf16"


def kernel(agent_hiddens, padding_mask, in_proj_weight, in_proj_bias,
           out_proj_weight, out_proj_bias):
    in_maps, meta = make_host_inputs(
        agent_hiddens, padding_mask, in_proj_weight, in_proj_bias,
        out_proj_weight, out_proj_bias, mm_dtype=MM_DTYPE)
    nc = _get_nc(dict(b_core=meta["b_core"], mm_dtype=MM_DTYPE, reps=1,
                      with_pad=meta["with_pad"], with_bias=meta["with_bias"]))
    res = run_bass_kernel_spmd(nc, in_maps, list(range(N_CORES)))
    return assemble_output(res.results, meta["b_core"])



# revision 47
# speedup vs baseline: 1.0468x; 1.0468x over previous
"""CrossAgentAttention Trainium2 kernel.

Problem: B=1024 samples, N=32 agents, D=512 features, H=4 heads (HD=128).
  qkv = x @ Win^T + bin ; per-head attention over the N=32 agents with the
  diagonal (self) and padded agents masked out of the keys; out = ctx @ Wout^T + bout.

Strategy (data-parallel over B across 8 cores, weights replicated):
  - Host pre-transposes the per-core activations to X^T [D, T] (T = B/8*32 tokens)
    and the weights to Win^T / Wout^T so every GEMM contraction dim lands on
    SBUF partitions.  Q columns of Win^T are pre-scaled by 1/sqrt(HD).
  - Stage 1: Q^T,K^T [1024, T] in transposed (feature-major) layout and
    V [T, 512] token-major, via f32r matmuls with N=512 moving operands.
  - Stage 2: attention per (sample-group of 4, head).  128 tokens = 4 samples
    x 32 agents: S = Q^T.T @ K^T gives all 16 cross-sample blocks; an additive
    mask kills cross-sample blocks, the self-diagonal, and padded keys.
    Softmax without max-subtraction (logits are O(1) by construction),
    exp+rowsum fused on ACT, P normalized on DVE, P^T via PE transpose,
    ctx^T = (V slice).T @ P^T.
  - Stage 3: OUT^T = Wout^T.T @ ctx^T, DMA out; host transposes back.
"""

import math

import ml_dtypes
import numpy as np

import concourse.bass as bass
import concourse.mybir as mybir
import concourse.tile as tile
from concourse import bacc
from concourse.bass_utils import run_bass_kernel_spmd

N_CORES = 8
B, N, D, H = 1024, 32, 512, 4
HD = D // H  # 128
NEG = -60000.0  # additive mask value; exp() underflows to exactly 0.0
F32 = mybir.dt.float32
F32R = mybir.dt.float32r
BF16 = mybir.dt.bfloat16

_MM_DT = {"f32r": F32R, "f32": F32, "bf16": BF16}


def build_program(b_core, mm_dtype="bf16", reps=1, with_pad=False,
                  with_bias=False, skel=False):
    """Trace + compile the per-core program. Returns (nc, meta)."""
    T = b_core * N  # tokens per core
    GT = 512 if T >= 512 else T  # tokens per group
    G = T // GT  # groups
    TT = GT // 128  # 128-token tiles (sample groups of 4) per group
    assert T % 512 == 0 or G == 1

    nc = bacc.Bacc("TRN2", target_bir_lowering=False, debug=False, num_devices=N_CORES)

    MD = _MM_DT[mm_dtype]  # matmul-operand dtype
    OD = BF16 if mm_dtype == "bf16" else F32  # DRAM output dtype
    # [g, p, k, c] = X^T[k*128+p, g*GT+c]; one contiguous DMA per group
    # (4*GT*elem per partition line) instead of 4 strided 128-line loads.
    xt = nc.dram_tensor("xt", [G, 128, 4, GT], MD, kind="ExternalInput").ap()
    wint = nc.dram_tensor("wint", [D, 3 * D], MD, kind="ExternalInput").ap()
    woutt = nc.dram_tensor("woutt", [D, D], MD, kind="ExternalInput").ap()
    ident = nc.dram_tensor("ident", [128, 128], MD, kind="ExternalInput").ap()
    if with_pad:
        mask = nc.dram_tensor("mask", [T // 128, 128, 4 * 128], F32,
                              kind="ExternalInput").ap()
    else:
        mask = nc.dram_tensor("mask", [128, TT, 4 * 128], F32,
                              kind="ExternalInput").ap()
    if with_bias:
        bqk = nc.dram_tensor("bqk", [128, 8], F32, kind="ExternalInput").ap()
        bv = nc.dram_tensor("bv", [1, D], MD, kind="ExternalInput").ap()
        bo = nc.dram_tensor("bo", [128, 4], F32, kind="ExternalInput").ap()
    # [g, p, fo, c] = OUT^T[fo*128+p, g*GT+c]; one contiguous DMA per group
    outt = nc.dram_tensor("outt", [G, 128, 4, GT], OD, kind="ExternalOutput").ap()

    assert TT % 2 == 0
    NP = TT // 2  # attention tile-pairs per group

    with tile.TileContext(nc) as tc:
        with (
            tc.tile_pool(name="wpool", bufs=1) as wpool,
            tc.tile_pool(name="xtp", bufs=2, space="SBUF") as xtp,
            tc.tile_pool(name="qktp", bufs=2 * 8) as qktp,
            tc.tile_pool(name="vp", bufs=2 * TT) as vp,
            tc.tile_pool(name="smp", bufs=2) as smp,
            tc.tile_pool(name="ctxp", bufs=2) as ctxp,
            tc.tile_pool(name="otp", bufs=2) as otp,
            tc.tile_pool(name="mmps", bufs=3, space="PSUM") as mmps,
            tc.tile_pool(name="spsp", bufs=1, space="PSUM") as spsp,
            tc.tile_pool(name="ptpsp", bufs=1, space="PSUM") as ptpsp,
            tc.tile_pool(name="cpsp", bufs=1, space="PSUM") as cpsp,
        ):
            # ---- resident weights / constants ----
            w = []
            for k in range(4):
                wt = wpool.tile([128, 3 * D], MD, tag=f"wint{k}")
                w.append(wt)
            # chunked so Q columns (chunk 0) land first; K then V follow
            for c in range(3):
                for k in range(4):
                    nc.sync.dma_start(
                        w[k][:, bass.ts(c, D)],
                        wint[k * 128:(k + 1) * 128, bass.ts(c, D)])
            idt = wpool.tile([128, 128], MD, tag="ident")
            nc.sync.dma_start(idt[:], ident[:])
            mk_const = None
            if not with_pad:
                mk_const = wpool.tile([128, TT, 4 * 128], F32, tag="mask")
                nc.sync.dma_start(mk_const[:], mask[:])
            wo = []
            for k in range(4):
                wt = wpool.tile([128, D], MD, tag=f"woutt{k}")
                nc.sync.dma_start(wt[:], woutt[k * 128:(k + 1) * 128, :])
                wo.append(wt)
            if with_bias:
                bqk_sb = wpool.tile([128, 8], F32, tag="bqk")
                nc.sync.dma_start(bqk_sb[:], bqk[:])
                bv_sb = wpool.tile([1, D], MD, tag="bv")
                nc.sync.dma_start(bv_sb[:], bv[:])
                bo_sb = wpool.tile([128, 4], F32, tag="bo")
                nc.sync.dma_start(bo_sb[:], bo[:])
                ones_sb = wpool.tile([1, 128], MD, tag="ones")
                nc.vector.memset(ones_sb[:], 1.0)

            def body(_iv=None):
                def emit_load(g):
                    xgt = xtp.tile([128, 4, GT], MD, tag="xt", name="xgt")
                    nc.scalar.dma_start(xgt[:], xt[g])
                    return xgt

                def emit_1a(xgt):
                    # Q^T, K^T (feature-major)
                    qkt = []
                    for fo in range(8):
                        ps = mmps.tile([128, GT], F32, tag="mm", name="ps")
                        for k in range(4):
                            nc.tensor.matmul(
                                ps[:],
                                w[k][:, bass.ts(fo, 128)],
                                xgt[:, k, :],
                                start=(k == 0), stop=(k == 3),
                            )
                        qt = qktp.tile([128, GT], MD, tag="qkt", name="qt")
                        if with_bias:
                            nc.scalar.activation(
                                qt[:], ps[:],
                                mybir.ActivationFunctionType.Identity,
                                bias=bqk_sb[:, fo:fo + 1])
                        else:
                            nc.vector.tensor_copy(qt[:], ps[:])
                        qkt.append(qt)
                    return qkt

                def emit_1b_unit(xgt, tt):
                    # V (token-major), one 128-token tile
                    ps = mmps.tile([128, D], F32, tag="mm", name="ps")
                    for k in range(4):
                        nc.tensor.matmul(
                            ps[:],
                            xgt[:, k, bass.ts(tt, 128)],
                            w[k][:, 2 * D:3 * D],
                            start=(k == 0),
                            stop=(k == 3 and not with_bias),
                        )
                    if with_bias:
                        nc.tensor.matmul(
                            ps[:], ones_sb[:], bv_sb[:],
                            start=False, stop=True,
                        )
                    vt = vp.tile([128, D], MD, tag="v", name="vt")
                    nc.scalar.copy(vt[:], ps[:])
                    return vt

                def emit_1b(xgt):
                    return [emit_1b_unit(xgt, tt) for tt in range(TT)]

                ot_holder = [None]

                def emit_outproj(ctxt_prev, g_prev, half=None):
                    halves = (0, 1) if half is None else (half,)
                    for hf in halves:
                        if hf == 0:
                            ot_holder[0] = otp.tile([128, 4, GT], OD,
                                                    tag="ot", name="ot")
                        ot = ot_holder[0]
                        for fo in (2 * hf, 2 * hf + 1):
                            ps = mmps.tile([128, GT], F32, tag="mm", name="ps")
                            for k in range(4):
                                nc.tensor.matmul(
                                    ps[:],
                                    wo[k][:, bass.ts(fo, 128)],
                                    ctxt_prev[:, k, :],
                                    start=(k == 0), stop=(k == 3),
                                )
                            if with_bias:
                                nc.scalar.activation(
                                    ot[:, fo, :], ps[:],
                                    mybir.ActivationFunctionType.Identity,
                                    bias=bo_sb[:, fo:fo + 1])
                            else:
                                nc.scalar.copy(ot[:, fo, :], ps[:])
                        if hf == 1:
                            nc.sync.dma_start(outt[g_prev], ot[:])

                if skel:
                    pending = None
                    for g in range(G):
                        xgt = emit_load(g)
                        qkt = emit_1a(xgt)
                        emit_1b(xgt)
                        if pending is not None:
                            emit_outproj(*pending)
                        ctxt = ctxp.tile([128, 4, GT], MD, tag="ctxt",
                                         name="ctxt")
                        for k in range(4):
                            nc.vector.tensor_copy(ctxt[:, k, :], qkt[k][:])
                        pending = (ctxt, g)
                    emit_outproj(*pending)
                    return

                # ---- attention per pair of 128-token tiles; exp(S+M) ==
                # exp(S)*Z with Z in {0,1}: exp reads raw S from PSUM, mask
                # folded into the DVE pipeline as a multiply. Group g+1's
                # QKV projections are emitted inside group g's attention as
                # PE fill while the softmax chains drain on ACT/DVE.
                pnbs, spss, zmasks, ptsbs = {}, {}, {}, {}

                def stA(g, u, qkt):
                    sps = spsp.tile([128, 2, 4, 128], F32, tag="sps",
                                    name="sps")
                    if with_pad:
                        mk = smp.tile([128, 2, 4 * 128], F32, tag="mask",
                                      name="mk")
                        for i in range(2):
                            nc.sync.dma_start(
                                mk[:, i, :], mask[g * TT + 2 * u + i])
                        zmasks[u] = mk[:]
                    else:
                        zmasks[u] = mk_const[:, 2 * u:2 * u + 2, :]
                    for i in range(2):
                        tsl = bass.ts(2 * u + i, 128)
                        for h in range(4):
                            nc.tensor.matmul(
                                sps[:, i, h, :],
                                qkt[h][:, tsl],
                                qkt[4 + h][:, tsl],
                                start=True, stop=True,
                            )
                    spss[u] = sps

                def stSM(u):
                    sps = spss.pop(u)
                    zm = zmasks.pop(u)
                    psb = smp.tile([128, 2, 4, 128], F32, tag="psb",
                                   name="psb")
                    nc.scalar.activation(
                        psb[:], sps[:], mybir.ActivationFunctionType.Exp)
                    pz = smp.tile([128, 2, 4, 128], F32, tag="pz", name="pz")
                    nc.vector.tensor_mul(
                        pz[:].rearrange("p t h j -> p t (h j)"),
                        psb[:].rearrange("p t h j -> p t (h j)"),
                        zm)
                    rsum = smp.tile([128, 16], F32, tag="rsum", name="rsum")
                    nc.vector.reduce_sum(
                        rsum[:, 0:8],
                        pz[:].rearrange("p t h j -> p (t h) j"),
                        axis=mybir.AxisListType.X)
                    nc.vector.reciprocal(rsum[:, 8:16], rsum[:, 0:8])
                    pnb = smp.tile([128, 2, 4, 128], MD, tag="pnb",
                                   name="pnb")
                    rb = rsum[:, 8:16]
                    rinv_b = bass.AP(tensor=rb.tensor, offset=rb.offset,
                                     ap=list(rb.ap) + [[0, 128]])
                    nc.vector.tensor_mul(
                        pnb[:].rearrange("p t h j -> p (t h) j"),
                        pz[:].rearrange("p t h j -> p (t h) j"),
                        rinv_b)
                    pnbs[u] = pnb

                def stB(u):
                    pnb = pnbs.pop(u)
                    ptps = ptpsp.tile([128, 2, 4, 128], MD, tag="ptps",
                                      name="ptps")
                    for i in range(2):
                        for h in range(4):
                            nc.tensor.transpose(
                                ptps[:, i, h, :], pnb[:, i, h, :], idt[:])
                    ptsb = smp.tile([128, 2, 4, 128], MD, tag="ptsb",
                                    name="ptsb")
                    nc.scalar.copy(ptsb[:], ptps[:])
                    ptsbs[u] = ptsb

                def stC(u, ctxt, vg):
                    ptsb = ptsbs.pop(u)
                    cps = cpsp.tile([128, 2, 4, 128], F32, tag="cps",
                                    name="cps")
                    for i in range(2):
                        tt = 2 * u + i
                        for h in range(4):
                            nc.tensor.matmul(
                                cps[:, i, h, :],
                                vg[tt][:, bass.ts(h, 128)],
                                ptsb[:, i, h, :],
                                start=True, stop=True,
                            )
                    nc.scalar.copy(
                        ctxt[:, :, 2 * u * 128:(2 * u + 2) * 128]
                            .rearrange("p h (t j) -> p h t j", t=2),
                        cps[:].rearrange("p t h j -> p h t j"))

                # prologue: first group's projections
                xgt = emit_load(0)
                qkts = {0: emit_1a(xgt)}
                vgs = {0: emit_1b(xgt)}
                pending = None
                for g in range(G):
                    last = g + 1 >= G
                    if not last:
                        xgt = emit_load(g + 1)
                    qkt = qkts.pop(g)
                    vg = vgs.pop(g)
                    ctxt = ctxp.tile([128, 4, GT], MD, tag="ctxt", name="ctxt")
                    stA(g, 0, qkt)
                    stSM(0)
                    if pending is not None:
                        emit_outproj(*pending, half=0)
                    stA(g, 1, qkt)
                    stSM(1)
                    if not last:
                        qkts[g + 1] = emit_1a(xgt)  # fill: softmax chains
                    stB(0)
                    if not last:
                        vgs[g + 1] = [emit_1b_unit(xgt, 0),
                                      emit_1b_unit(xgt, 1)]
                    stB(1)
                    if pending is not None:
                        emit_outproj(*pending, half=1)
                    stC(0, ctxt, vg)
                    if not last:
                        vgs[g + 1] += [emit_1b_unit(xgt, 2),
                                       emit_1b_unit(xgt, 3)]
                    stC(1, ctxt, vg)
                    pending = (ctxt, g)
                emit_outproj(*pending)

            if reps == 1:
                body()
            else:
                with tc.For_i(0, reps, 1, hint_engines=(
                        mybir.EngineType.PE, mybir.EngineType.DVE,
                        mybir.EngineType.Activation, mybir.EngineType.SP)) as iv:
                    body(iv)

    nc.compile()
    return nc


def make_host_inputs(agent_hiddens, padding_mask, in_proj_weight, in_proj_bias,
                     out_proj_weight, out_proj_bias, mm_dtype="bf16"):
    """Shard + lay out host-side numpy arrays. Returns (in_maps, flags)."""
    hd = ml_dtypes.bfloat16 if mm_dtype == "bf16" else np.float32
    x = np.asarray(agent_hiddens, dtype=np.float32)
    pad = np.asarray(padding_mask)
    win = np.asarray(in_proj_weight, dtype=np.float32)
    bin_ = np.asarray(in_proj_bias, dtype=np.float32)
    wout = np.asarray(out_proj_weight, dtype=np.float32)
    bout = np.asarray(out_proj_bias, dtype=np.float32)

    b = x.shape[0]
    b_core = b // N_CORES
    T = b_core * N
    scale = 1.0 / math.sqrt(HD)

    with_pad = bool(pad.any())
    with_bias = bool(bin_.any() or bout.any())

    wint = np.ascontiguousarray(win.T)
    wint[:, :D] *= scale
    wint = wint.astype(hd)
    woutt = np.ascontiguousarray(wout.T).astype(hd)
    identity = np.eye(128, dtype=hd)

    # 128-token block mask: tokens (s, i) x (s', j); mask cross-sample blocks
    # and the global diagonal (self-attention). The kernel consumes 0/1
    # multiplicative masks (applied after exp).
    p = np.arange(128)
    blocked = ((p[:, None] // 32 != p[None, :] // 32)
               | (p[:, None] == p[None, :]))
    mask_tile = np.tile(np.where(blocked, 0.0, 1.0).astype(np.float32),
                        (1, 4))  # [128, 512], head-replicated

    G = T // 512 if T >= 512 else 1
    GT = T // G
    in_maps = []
    for c in range(N_CORES):
        xc = x[c * b_core:(c + 1) * b_core].reshape(T, D)
        # [G, 128, 4, GT]: [g, p, k, c] = X^T[k*128+p, g*GT+c]
        xct = xc.T.reshape(4, 128, G, GT).transpose(2, 1, 0, 3)
        m = {
            "xt": np.ascontiguousarray(xct).astype(hd),
            "wint": wint,
            "woutt": woutt,
            "ident": identity,
        }
        if with_pad:
            padc = pad[c * b_core:(c + 1) * b_core]  # [b_core, N]
            n_tt = T // 128
            mt = np.empty((n_tt, 128, 512), dtype=np.float32)
            for t in range(n_tt):
                # 4 samples in this tile; key-padding kills columns
                pr = padc[t * 4:(t + 1) * 4].reshape(128)  # [(s', j)] order
                keep = (~blocked) & (~pr[None, :])
                mt[t] = np.tile(keep.astype(np.float32), (1, 4))
            m["mask"] = mt
        else:
            # [128, TT, 512]: same per-tile mask for every tile of a group
            TT = GT // 128
            m["mask"] = np.ascontiguousarray(
                np.broadcast_to(mask_tile[:, None, :], (128, TT, 512)))
        if with_bias:
            bq = bin_[:D] * scale
            bk = bin_[D:2 * D]
            m["bqk"] = np.ascontiguousarray(
                np.concatenate([bq, bk]).reshape(8, 128).T)
            m["bv"] = bin_[2 * D:3 * D].reshape(1, D).astype(hd)
            m["bo"] = np.ascontiguousarray(bout.reshape(4, 128).T)
        in_maps.append(m)
    return in_maps, dict(b_core=b_core, with_pad=with_pad, with_bias=with_bias)


def assemble_output(results, b_core):
    outs = []
    for c in range(N_CORES):
        ot = np.asarray(results[c]["outt"], dtype=np.float32)  # [G,128,4,GT]
        # [g, p, fo, c] = OUT^T[fo*128+p, g*GT+c] -> [token, feat]
        G = ot.shape[0]
        out = ot.transpose(0, 3, 2, 1).reshape(b_core * N, D)
        outs.append(out.reshape(b_core, N, D))
    return np.ascontiguousarray(np.concatenate(outs, axis=0))


_NC_CACHE = {}


def _get_nc(key_args):
    key = tuple(sorted(key_args.items()))
    if key not in _NC_CACHE:
        _NC_CACHE[key] = build_program(**key_args)
    return _NC_CACHE[key]


MM_DTYPE = "bf16"


def kernel(agent_hiddens, padding_mask, in_proj_weight, in_proj_bias,
           out_proj_weight, out_proj_bias):
    in_maps, meta = make_host_inputs(
        agent_hiddens, padding_mask, in_proj_weight, in_proj_bias,
        out_proj_weight, out_proj_bias, mm_dtype=MM_DTYPE)
    nc = _get_nc(dict(b_core=meta["b_core"], mm_dtype=MM_DTYPE, reps=1,
                      with_pad=meta["with_pad"], with_bias=meta["with_bias"]))
    res = run_bass_kernel_spmd(nc, in_maps, list(range(N_CORES)))
    return assemble_output(res.results, meta["b_core"])



# revision 48
# speedup vs baseline: 1.1786x; 1.1259x over previous
"""CrossAgentAttention Trainium2 kernel.

Problem: B=1024 samples, N=32 agents, D=512 features, H=4 heads (HD=128).
  qkv = x @ Win^T + bin ; per-head attention over the N=32 agents with the
  diagonal (self) and padded agents masked out of the keys; out = ctx @ Wout^T + bout.

Strategy (data-parallel over B across 8 cores, weights replicated):
  - Host pre-transposes the per-core activations to X^T [D, T] (T = B/8*32 tokens)
    and the weights to Win^T / Wout^T so every GEMM contraction dim lands on
    SBUF partitions.  Q columns of Win^T are pre-scaled by 1/sqrt(HD).
  - Stage 1: Q^T,K^T [1024, T] in transposed (feature-major) layout and
    V [T, 512] token-major, via f32r matmuls with N=512 moving operands.
  - Stage 2: attention per (sample-group of 4, head).  128 tokens = 4 samples
    x 32 agents: S = Q^T.T @ K^T gives all 16 cross-sample blocks; an additive
    mask kills cross-sample blocks, the self-diagonal, and padded keys.
    Softmax without max-subtraction (logits are O(1) by construction),
    exp+rowsum fused on ACT, P normalized on DVE, P^T via PE transpose,
    ctx^T = (V slice).T @ P^T.
  - Stage 3: OUT^T = Wout^T.T @ ctx^T, DMA out; host transposes back.
"""

import math

import ml_dtypes
import numpy as np

import concourse.bass as bass
import concourse.mybir as mybir
import concourse.tile as tile
from concourse import bacc
from concourse.bass_utils import run_bass_kernel_spmd

N_CORES = 8
B, N, D, H = 1024, 32, 512, 4
HD = D // H  # 128
NEG = -60000.0  # additive mask value; exp() underflows to exactly 0.0
F32 = mybir.dt.float32
F32R = mybir.dt.float32r
BF16 = mybir.dt.bfloat16

_MM_DT = {"f32r": F32R, "f32": F32, "bf16": BF16}


def build_program(b_core, mm_dtype="bf16", reps=1, with_pad=False,
                  with_bias=False, skel=False):
    """Trace + compile the per-core program. Returns (nc, meta)."""
    T = b_core * N  # tokens per core
    GT = 512 if T >= 512 else T  # tokens per group
    G = T // GT  # groups
    TT = GT // 128  # 128-token tiles (sample groups of 4) per group
    assert T % 512 == 0 or G == 1

    nc = bacc.Bacc("TRN2", target_bir_lowering=False, debug=False, num_devices=N_CORES)

    MD = _MM_DT[mm_dtype]  # matmul-operand dtype
    OD = BF16 if mm_dtype == "bf16" else F32  # DRAM output dtype
    # [g, p, k, c] = X^T[k*128+p, g*GT+c]; one contiguous DMA per group
    # (4*GT*elem per partition line) instead of 4 strided 128-line loads.
    xt = nc.dram_tensor("xt", [G, 128, 4, GT], MD, kind="ExternalInput").ap()
    wint = nc.dram_tensor("wint", [D, 3 * D], MD, kind="ExternalInput").ap()
    woutt = nc.dram_tensor("woutt", [D, D], MD, kind="ExternalInput").ap()
    ident = nc.dram_tensor("ident", [128, 128], MD, kind="ExternalInput").ap()
    if with_pad:
        mask = nc.dram_tensor("mask", [T // 128, 128, 4 * 128], F32,
                              kind="ExternalInput").ap()
    else:
        mask = nc.dram_tensor("mask", [128, TT, 4 * 128], F32,
                              kind="ExternalInput").ap()
    if with_bias:
        bqk = nc.dram_tensor("bqk", [128, 8], F32, kind="ExternalInput").ap()
        bv = nc.dram_tensor("bv", [1, D], MD, kind="ExternalInput").ap()
        bo = nc.dram_tensor("bo", [128, 4], F32, kind="ExternalInput").ap()
    # [g, p, fo, c] = OUT^T[fo*128+p, g*GT+c]; one contiguous DMA per group
    outt = nc.dram_tensor("outt", [G, 128, 4, GT], OD, kind="ExternalOutput").ap()

    assert TT % 2 == 0
    NP = TT // 2  # attention tile-pairs per group

    with tile.TileContext(nc) as tc:
        with (
            tc.tile_pool(name="wpool", bufs=1) as wpool,
            tc.tile_pool(name="xtp", bufs=2, space="SBUF") as xtp,
            tc.tile_pool(name="qktp", bufs=2 * 8) as qktp,
            tc.tile_pool(name="vp", bufs=2 * TT) as vp,
            tc.tile_pool(name="smp", bufs=2) as smp,
            tc.tile_pool(name="ctxp", bufs=2) as ctxp,
            tc.tile_pool(name="otp", bufs=2) as otp,
            tc.tile_pool(name="mmps", bufs=3, space="PSUM") as mmps,
            tc.tile_pool(name="spsp", bufs=1, space="PSUM") as spsp,
            tc.tile_pool(name="ptpsp", bufs=1, space="PSUM") as ptpsp,
            tc.tile_pool(name="cpsp", bufs=1, space="PSUM") as cpsp,
        ):
            # ---- resident weights / constants ----
            w = []
            for k in range(4):
                wt = wpool.tile([128, 3 * D], MD, tag=f"wint{k}")
                w.append(wt)
            # chunked so Q columns (chunk 0) land first; K then V follow
            for c in range(3):
                for k in range(4):
                    nc.sync.dma_start(
                        w[k][:, bass.ts(c, D)],
                        wint[k * 128:(k + 1) * 128, bass.ts(c, D)])
            idt = wpool.tile([128, 128], MD, tag="ident")
            nc.sync.dma_start(idt[:], ident[:])
            mk_const = None
            if not with_pad:
                mk_const = wpool.tile([128, TT, 4 * 128], F32, tag="mask")
                nc.sync.dma_start(mk_const[:], mask[:])
            wo = []
            for k in range(4):
                wt = wpool.tile([128, D], MD, tag=f"woutt{k}")
                nc.sync.dma_start(wt[:], woutt[k * 128:(k + 1) * 128, :])
                wo.append(wt)
            if with_bias:
                bqk_sb = wpool.tile([128, 8], F32, tag="bqk")
                nc.sync.dma_start(bqk_sb[:], bqk[:])
                bv_sb = wpool.tile([1, D], MD, tag="bv")
                nc.sync.dma_start(bv_sb[:], bv[:])
                bo_sb = wpool.tile([128, 4], F32, tag="bo")
                nc.sync.dma_start(bo_sb[:], bo[:])
                ones_sb = wpool.tile([1, 128], MD, tag="ones")
                nc.vector.memset(ones_sb[:], 1.0)

            def body(_iv=None):
                def emit_load(g):
                    xgt = xtp.tile([128, 4, GT], MD, tag="xt", name="xgt")
                    nc.scalar.dma_start(xgt[:], xt[g])
                    return xgt

                def emit_1a(xgt):
                    # Q^T, K^T (feature-major)
                    qkt = []
                    for fo in range(8):
                        ps = mmps.tile([128, GT], F32, tag="mm", name="ps")
                        for k in range(4):
                            nc.tensor.matmul(
                                ps[:],
                                w[k][:, bass.ts(fo, 128)],
                                xgt[:, k, :],
                                start=(k == 0), stop=(k == 3),
                            )
                        qt = qktp.tile([128, GT], MD, tag="qkt", name="qt")
                        if with_bias:
                            nc.scalar.activation(
                                qt[:], ps[:],
                                mybir.ActivationFunctionType.Identity,
                                bias=bqk_sb[:, fo:fo + 1])
                        else:
                            nc.vector.tensor_copy(qt[:], ps[:])
                        qkt.append(qt)
                    return qkt

                def emit_1b_unit(xgt, tt):
                    # V (token-major), one 128-token tile
                    ps = mmps.tile([128, D], F32, tag="mm", name="ps")
                    for k in range(4):
                        nc.tensor.matmul(
                            ps[:],
                            xgt[:, k, bass.ts(tt, 128)],
                            w[k][:, 2 * D:3 * D],
                            start=(k == 0),
                            stop=(k == 3 and not with_bias),
                        )
                    if with_bias:
                        nc.tensor.matmul(
                            ps[:], ones_sb[:], bv_sb[:],
                            start=False, stop=True,
                        )
                    vt = vp.tile([128, D], MD, tag="v", name="vt")
                    nc.scalar.copy(vt[:], ps[:])
                    return vt

                def emit_1b(xgt):
                    return [emit_1b_unit(xgt, tt) for tt in range(TT)]

                ot_holder = [None]

                def emit_outproj(ctxt_prev, g_prev, half=None):
                    halves = (0, 1) if half is None else (half,)
                    for hf in halves:
                        if hf == 0:
                            ot_holder[0] = otp.tile([128, 4, GT], OD,
                                                    tag="ot", name="ot")
                        ot = ot_holder[0]
                        for fo in (2 * hf, 2 * hf + 1):
                            ps = mmps.tile([128, GT], F32, tag="mm", name="ps")
                            for k in range(4):
                                nc.tensor.matmul(
                                    ps[:],
                                    wo[k][:, bass.ts(fo, 128)],
                                    ctxt_prev[:, k, :],
                                    start=(k == 0), stop=(k == 3),
                                )
                            if with_bias:
                                nc.scalar.activation(
                                    ot[:, fo, :], ps[:],
                                    mybir.ActivationFunctionType.Identity,
                                    bias=bo_sb[:, fo:fo + 1])
                            else:
                                nc.scalar.copy(ot[:, fo, :], ps[:])
                        if hf == 1:
                            nc.sync.dma_start(outt[g_prev], ot[:])

                if skel:
                    pending = None
                    for g in range(G):
                        xgt = emit_load(g)
                        qkt = emit_1a(xgt)
                        emit_1b(xgt)
                        if pending is not None:
                            emit_outproj(*pending)
                        ctxt = ctxp.tile([128, 4, GT], MD, tag="ctxt",
                                         name="ctxt")
                        for k in range(4):
                            nc.vector.tensor_copy(ctxt[:, k, :], qkt[k][:])
                        pending = (ctxt, g)
                    emit_outproj(*pending)
                    return

                # ---- attention per pair of 128-token tiles; exp(S+M) ==
                # exp(S)*Z with Z in {0,1}: exp reads raw S from PSUM, mask
                # folded into the DVE pipeline as a multiply. Group g+1's
                # QKV projections are emitted inside group g's attention as
                # PE fill while the softmax chains drain on ACT/DVE.
                pnbs, spss, zmasks, ptsbs = {}, {}, {}, {}

                def stA(g, u, qkt):
                    sps = spsp.tile([128, 2, 4, 128], F32, tag="sps",
                                    name="sps")
                    if with_pad:
                        mk = smp.tile([128, 2, 4 * 128], F32, tag="mask",
                                      name="mk")
                        for i in range(2):
                            nc.sync.dma_start(
                                mk[:, i, :], mask[g * TT + 2 * u + i])
                        zmasks[u] = mk[:]
                    else:
                        zmasks[u] = mk_const[:, 2 * u:2 * u + 2, :]
                    for i in range(2):
                        tsl = bass.ts(2 * u + i, 128)
                        for h in range(4):
                            nc.tensor.matmul(
                                sps[:, i, h, :],
                                qkt[h][:, tsl],
                                qkt[4 + h][:, tsl],
                                start=True, stop=True,
                            )
                    spss[u] = sps

                def stSM(u):
                    sps = spss.pop(u)
                    zm = zmasks.pop(u)
                    psb = smp.tile([128, 2, 4, 128], F32, tag="psb",
                                   name="psb")
                    nc.scalar.activation(
                        psb[:], sps[:], mybir.ActivationFunctionType.Exp)
                    pz = smp.tile([128, 2, 4, 128], F32, tag="pz", name="pz")
                    nc.vector.tensor_mul(
                        pz[:].rearrange("p t h j -> p t (h j)"),
                        psb[:].rearrange("p t h j -> p t (h j)"),
                        zm)
                    rsum = smp.tile([128, 16], F32, tag="rsum", name="rsum")
                    nc.vector.reduce_sum(
                        rsum[:, 0:8],
                        pz[:].rearrange("p t h j -> p (t h) j"),
                        axis=mybir.AxisListType.X)
                    nc.vector.reciprocal(rsum[:, 8:16], rsum[:, 0:8])
                    pnb = smp.tile([128, 2, 4, 128], MD, tag="pnb",
                                   name="pnb")
                    rb = rsum[:, 8:16]
                    rinv_b = bass.AP(tensor=rb.tensor, offset=rb.offset,
                                     ap=list(rb.ap) + [[0, 128]])
                    nc.vector.tensor_mul(
                        pnb[:].rearrange("p t h j -> p (t h) j"),
                        pz[:].rearrange("p t h j -> p (t h) j"),
                        rinv_b)
                    pnbs[u] = pnb

                def stB(u):
                    pnb = pnbs.pop(u)
                    ptps = ptpsp.tile([128, 2, 4, 128], MD, tag="ptps",
                                      name="ptps")
                    for i in range(2):
                        for h in range(4):
                            nc.tensor.transpose(
                                ptps[:, i, h, :], pnb[:, i, h, :], idt[:])
                    ptsb = smp.tile([128, 2, 4, 128], MD, tag="ptsb",
                                    name="ptsb")
                    nc.scalar.copy(ptsb[:], ptps[:])
                    ptsbs[u] = ptsb

                def stC(u, ctxt, vg):
                    ptsb = ptsbs.pop(u)
                    cps = cpsp.tile([128, 2, 4, 128], F32, tag="cps",
                                    name="cps")
                    for i in range(2):
                        tt = 2 * u + i
                        for h in range(4):
                            nc.tensor.matmul(
                                cps[:, i, h, :],
                                vg[tt][:, bass.ts(h, 128)],
                                ptsb[:, i, h, :],
                                start=True, stop=True,
                            )
                    nc.scalar.copy(
                        ctxt[:, :, 2 * u * 128:(2 * u + 2) * 128]
                            .rearrange("p h (t j) -> p h t j", t=2),
                        cps[:].rearrange("p t h j -> p h t j"))

                # schedule: same-group projections, attention with
                # outproj(g-1) as PE fill; X^T loads prefetched one group
                # ahead so 1a never waits on the DMA.
                xgts = {0: emit_load(0)}
                pending = None
                for g in range(G):
                    if g + 1 < G:
                        xgts[g + 1] = emit_load(g + 1)
                    xgt = xgts.pop(g)
                    qkt = emit_1a(xgt)
                    vg = emit_1b(xgt)
                    ctxt = ctxp.tile([128, 4, GT], MD, tag="ctxt", name="ctxt")
                    stA(g, 0, qkt)
                    stSM(0)
                    if pending is not None:
                        emit_outproj(*pending, half=0)
                    stA(g, 1, qkt)
                    stSM(1)
                    if pending is not None:
                        emit_outproj(*pending, half=1)
                    stB(0)
                    stC(0, ctxt, vg)
                    stB(1)
                    stC(1, ctxt, vg)
                    pending = (ctxt, g)
                emit_outproj(*pending)

            if reps == 1:
                body()
            else:
                with tc.For_i(0, reps, 1, hint_engines=(
                        mybir.EngineType.PE, mybir.EngineType.DVE,
                        mybir.EngineType.Activation, mybir.EngineType.SP)) as iv:
                    body(iv)

    nc.compile()
    return nc


def make_host_inputs(agent_hiddens, padding_mask, in_proj_weight, in_proj_bias,
                     out_proj_weight, out_proj_bias, mm_dtype="bf16"):
    """Shard + lay out host-side numpy arrays. Returns (in_maps, flags)."""
    hd = ml_dtypes.bfloat16 if mm_dtype == "bf16" else np.float32
    x = np.asarray(agent_hiddens, dtype=np.float32)
    pad = np.asarray(padding_mask)
    win = np.asarray(in_proj_weight, dtype=np.float32)
    bin_ = np.asarray(in_proj_bias, dtype=np.float32)
    wout = np.asarray(out_proj_weight, dtype=np.float32)
    bout = np.asarray(out_proj_bias, dtype=np.float32)

    b = x.shape[0]
    b_core = b // N_CORES
    T = b_core * N
    scale = 1.0 / math.sqrt(HD)

    with_pad = bool(pad.any())
    with_bias = bool(bin_.any() or bout.any())

    wint = np.ascontiguousarray(win.T)
    wint[:, :D] *= scale
    wint = wint.astype(hd)
    woutt = np.ascontiguousarray(wout.T).astype(hd)
    identity = np.eye(128, dtype=hd)

    # 128-token block mask: tokens (s, i) x (s', j); mask cross-sample blocks
    # and the global diagonal (self-attention). The kernel consumes 0/1
    # multiplicative masks (applied after exp).
    p = np.arange(128)
    blocked = ((p[:, None] // 32 != p[None, :] // 32)
               | (p[:, None] == p[None, :]))
    mask_tile = np.tile(np.where(blocked, 0.0, 1.0).astype(np.float32),
                        (1, 4))  # [128, 512], head-replicated

    G = T // 512 if T >= 512 else 1
    GT = T // G
    in_maps = []
    for c in range(N_CORES):
        xc = x[c * b_core:(c + 1) * b_core].reshape(T, D)
        # [G, 128, 4, GT]: [g, p, k, c] = X^T[k*128+p, g*GT+c]
        xct = xc.T.reshape(4, 128, G, GT).transpose(2, 1, 0, 3)
        m = {
            "xt": np.ascontiguousarray(xct).astype(hd),
            "wint": wint,
            "woutt": woutt,
            "ident": identity,
        }
        if with_pad:
            padc = pad[c * b_core:(c + 1) * b_core]  # [b_core, N]
            n_tt = T // 128
            mt = np.empty((n_tt, 128, 512), dtype=np.float32)
            for t in range(n_tt):
                # 4 samples in this tile; key-padding kills columns
                pr = padc[t * 4:(t + 1) * 4].reshape(128)  # [(s', j)] order
                keep = (~blocked) & (~pr[None, :])
                mt[t] = np.tile(keep.astype(np.float32), (1, 4))
            m["mask"] = mt
        else:
            # [128, TT, 512]: same per-tile mask for every tile of a group
            TT = GT // 128
            m["mask"] = np.ascontiguousarray(
                np.broadcast_to(mask_tile[:, None, :], (128, TT, 512)))
        if with_bias:
            bq = bin_[:D] * scale
            bk = bin_[D:2 * D]
            m["bqk"] = np.ascontiguousarray(
                np.concatenate([bq, bk]).reshape(8, 128).T)
            m["bv"] = bin_[2 * D:3 * D].reshape(1, D).astype(hd)
            m["bo"] = np.ascontiguousarray(bout.reshape(4, 128).T)
        in_maps.append(m)
    return in_maps, dict(b_core=b_core, with_pad=with_pad, with_bias=with_bias)


def assemble_output(results, b_core):
    outs = []
    for c in range(N_CORES):
        ot = np.asarray(results[c]["outt"], dtype=np.float32)  # [G,128,4,GT]
        # [g, p, fo, c] = OUT^T[fo*128+p, g*GT+c] -> [token, feat]
        G = ot.shape[0]
        out = ot.transpose(0, 3, 2, 1).reshape(b_core * N, D)
        outs.append(out.reshape(b_core, N, D))
    return np.ascontiguousarray(np.concatenate(outs, axis=0))


_NC_CACHE = {}


def _get_nc(key_args):
    key = tuple(sorted(key_args.items()))
    if key not in _NC_CACHE:
        _NC_CACHE[key] = build_program(**key_args)
    return _NC_CACHE[key]


MM_DTYPE = "bf16"


def kernel(agent_hiddens, padding_mask, in_proj_weight, in_proj_bias,
           out_proj_weight, out_proj_bias):
    in_maps, meta = make_host_inputs(
        agent_hiddens, padding_mask, in_proj_weight, in_proj_bias,
        out_proj_weight, out_proj_bias, mm_dtype=MM_DTYPE)
    nc = _get_nc(dict(b_core=meta["b_core"], mm_dtype=MM_DTYPE, reps=1,
                      with_pad=meta["with_pad"], with_bias=meta["with_bias"]))
    res = run_bass_kernel_spmd(nc, in_maps, list(range(N_CORES)))
    return assemble_output(res.results, meta["b_core"])

